# revision 22
# baseline (speedup 1.0000x reference)
"""DigitCaps dynamic-routing kernel for 8 TRN2 NeuronCores.

Math refactor (u_hat is NEVER materialized - it would be 189 MB):
  u_hat[b,r,c,d] = sum_i W[r,c,d,i] * u[b,r,i]
  softmax over r without max-subtraction (b_ij values are O(1)):
      c_ij[r,c,d] = exp(b[r,c,d]) / Z[c,d],  Z = sum_r exp(b)
  s'[b,c,d] = sum_{r,i} (exp(b) * W)[r,c,d,i] u[b,r,i],  s = s'/Z
  v = squash(s) = squash(y/z) computed WITHOUT the division:
      v = y^3 / ((z^2 + y^2) * (|y| + eps*z))        (y = s', z = Z)
  b += (1/B) sum_b t[b,r,c] v[b,c,d],  t[b,r,c] = sum_i (sum_d W) u[b,r,i]
       (t is iteration-invariant -> computed once, during AllReduce 0)

Sharding: routes (R=1152) split across 8 cores (144 each). Iterations 0 and 1
end in a fused AllReduce of (partial s' || partial Z); iteration 2 ends in a
ReduceScatter so each core only computes/outputs v for its own 32 batches
(host concatenates the disjoint shards - pure gather).

Schedule notes (v2):
  - Collective readback is split across three DMA queues (sync/scalar/gpsimd)
    so the 120KB payload lands in ~1us instead of ~2.5us.
  - A short burst of dummy matmuls (LDWEIGHTS reads the first readback slice,
    so they cannot start before the collective completes) keeps the PE busy
    through the readback+squash window: the HAM clock gate reaches K=8/8
    before the 60 b-update matmuls issue, halving their pacing.
  - squash runs split across DVE (y^2 -> denom -> recip -> final mul) and
    GPSIMD (|y| via abs_max, y*|y|), with |y| as a single abs_max op.
  - pe expand matmuls are emitted per (j, group) and pipelined between the
    per-group b-update matmuls, so the E*W rhs products (on GPSIMD) overlap
    the b-update phase and the s' matmul stream is not vector-paced.
  - stage copies go scalar (bh0) + vector (bh1) in parallel; the staging DMAs
    to the collective input split across two queues.
"""

import os
import numpy as np

B, R, C, D, I = 256, 1152, 10, 16, 8
CD = C * D                 # 160
NCORES = 8
RL = R // NCORES           # 144 routes per core
NCHUNK = RL * I // 128     # 9 K-chunks of 128
NG = 3                     # chunk groups of 3 (b_ij tile partition packing)
NITER = 3
EPS = 1e-5
GW = NG * (B + CD)         # columns per chunk-group input tile (1248)
RSP = 128 // NCORES        # 16 partitions per core after ReduceScatter
NWARM = 40                 # keep-warm dummy matmul pairs per collective wait

_CACHE = {}


def _build_program():
    from contextlib import ExitStack

    import concourse.bass as bass
    import concourse.bacc as bacc
    import concourse.mybir as mybir
    import concourse.tile as tile

    f32 = mybir.dt.float32
    bf16 = mybir.dt.bfloat16
    AF = mybir.ActivationFunctionType
    ALU = mybir.AluOpType

    nc = bacc.Bacc(None, num_devices=NCORES)

    # One fused host parameter (a single H2D feed per core keeps launch skew
    # across cores minimal) but THREE device-side DMAs over its column ranges
    # so the iter-0 matmuls pipeline with the load. The payload is bf16 -
    # numerically identical to casting on device (all matmul operands are
    # bf16 anyway) at half the H2D/DMA bytes. The 0/1 selector constants
    # (expand / route mask) are generated on-device with affine_select.
    # Layout: [g0 | g1 | g2], each g = [uT_k x3 | Wt_k x3].
    DW = NG * GW
    data_d = nc.declare_dram_parameter("data", [128, DW], bf16, isOutput=False)
    out_d = nc.declare_dram_parameter("out", [RSP, 2 * CD], f32, isOutput=True)

    rgroups = [list(range(NCORES))]

    with tile.TileContext(nc) as tc, ExitStack() as ctx:
        singles = ctx.enter_context(tc.tile_pool(name="singles", bufs=1))
        wcpool = ctx.enter_context(tc.tile_pool(name="wc", bufs=9))
        stpool = ctx.enter_context(tc.tile_pool(name="stage", bufs=2))
        work = ctx.enter_context(tc.tile_pool(name="work", bufs=8))
        ps_s = ctx.enter_context(tc.tile_pool(name="ps_s", bufs=1, space="PSUM"))
        ps_e = ctx.enter_context(tc.tile_pool(name="ps_e", bufs=3, space="PSUM"))
        ps_z = ctx.enter_context(tc.tile_pool(name="ps_z", bufs=1, space="PSUM"))
        ps_b = ctx.enter_context(tc.tile_pool(name="ps_b", bufs=2, space="PSUM"))
        dram = ctx.enter_context(tc.tile_pool(name="dram", bufs=1, space="DRAM"))

        # Collective payloads in bf16: halves the wire bytes and the
        # staging/readback DMAs (CCE reduces in bf16; ~0.4% noise, well
        # inside the tolerance).
        cc = []
        for it in range(NITER):
            w = 2 * CD if it == 0 else 3 * CD
            po = RSP if it == NITER - 1 else 128   # RS output is a partition shard
            ci = dram.tile([128, w], bf16, tag=f"cc_in{it}", name=f"cc_in{it}")
            co = dram.tile([po, w], bf16, tag=f"cc_out{it}", name=f"cc_out{it}")
            cc.append((ci, co, w))

        # ---- input DMAs (issued immediately; matmuls chase them) ----
        sb_g = []
        for g in range(NG):
            t = singles.tile([128, GW], bf16, tag=f"g{g}", name=f"sbg{g}")
            nc.sync.dma_start(out=t, in_=data_d[:, g * GW:(g + 1) * GW])
            sb_g.append(t)

        def wtb(k):
            g, j = k // NG, k % NG
            return sb_g[g][:, NG * B + j * CD:NG * B + (j + 1) * CD]

        def utb(k, bh):
            g, j = k // NG, k % NG
            return sb_g[g][:, j * B + bh * 128:j * B + (bh + 1) * 128]

        # ---- iteration 0 partial s' (c_ij uniform): straight to AllReduce ----
        # b-halves sequenced so bh0's staging DMA overlaps bh1's matmuls.
        ci0, co0, w0 = cc[0]
        stage0 = stpool.tile([128, w0], bf16, tag="stage", name="stage0")
        st = [ps_s.tile([128, CD], f32, tag=f"s{bh}", name=f"s0{bh}") for bh in range(2)]
        for bh in range(2):
            for k in range(NCHUNK):
                nc.tensor.matmul(
                    st[bh],
                    utb(k, bh),
                    wtb(k),
                    start=(k == 0), stop=(k == NCHUNK - 1),
                )
            nc.scalar.copy(out=stage0[:, bh * CD:(bh + 1) * CD], in_=st[bh])
        nc.sync.dma_start(out=ci0[:], in_=stage0)
        nc.gpsimd.collective_compute(
            "AllReduce", mybir.AluOpType.add,
            replica_groups=rgroups, ins=[ci0.opt()], outs=[co0.opt()],
        )

        # ---- prep work that hides under AllReduce 0 ----
        sb_ones = singles.tile([48, 128], bf16, tag="ones")
        nc.vector.memset(sb_ones, 1.0)

        # expand[q, j*128 + a*8 + b] = (q == 16j + a): the partition-broadcast
        # selector for the pe matmuls, generated on-device (GPSIMD, idle here).
        sb_exb = singles.tile([48, NG * 128], bf16, tag="exb")
        for j in range(NG):
            nc.gpsimd.affine_select(
                out=sb_exb[:, j * 128:(j + 1) * 128].rearrange(
                    "q (a b) -> q a b", b=8
                ),
                in_=sb_ones.rearrange("q (a b) -> q a b", b=8),
                pattern=[[-1, 16], [0, 8]],
                compare_op=mybir.AluOpType.is_equal,
                fill=0.0,
                base=-16 * j,
                channel_multiplier=1,
            )

        # Wd[(rp,i), (k,c)] = (1/B) * sum_d Wt   (t pre-scaled by 1/B here)
        sb_Wd = singles.tile([128, NCHUNK * C], f32, tag="Wd")
        for k in range(NCHUNK):
            nc.vector.reduce_sum(
                out=sb_Wd[:, k * C:(k + 1) * C],
                in_=wtb(k).rearrange("p (c d) -> p c d", d=D),
                axis=mybir.AxisListType.X,
            )
        nc.vector.tensor_scalar_mul(sb_Wd, sb_Wd, 1.0 / B)

        # Block-diagonal Wd for the t matmul:
        # Wdbd[p, k*CD + rp*C + c] = Wd[p,(k,c)] * (rp == p//8),
        # built as two affine_selects (p <= 8*rp+7 then p >= 8*rp) over a
        # zero-stride broadcast of Wd - no mask constant needed.
        sb_Wdbd = singles.tile([128, NCHUNK * CD], bf16, tag="Wdbd")
        sb_Wtmp = singles.tile([128, NCHUNK * CD], bf16, tag="Wtmp")
        for k in range(NCHUNK):
            wd_sl = sb_Wd[:, k * C:(k + 1) * C]
            wd_bk = bass.AP(
                tensor=wd_sl.tensor, offset=wd_sl.offset,
                ap=[wd_sl.ap[0], [0, 16], [1, C]],
            )
            nc.gpsimd.affine_select(
                out=sb_Wtmp[:, k * CD:(k + 1) * CD].rearrange(
                    "p (rp c) -> p rp c", c=C
                ),
                in_=wd_bk,
                pattern=[[8, 16], [0, C]],
                compare_op=mybir.AluOpType.is_ge,
                fill=0.0,
                base=7,
                channel_multiplier=-1,
            )
            nc.gpsimd.affine_select(
                out=sb_Wdbd[:, k * CD:(k + 1) * CD].rearrange(
                    "p (rp c) -> p rp c", c=C
                ),
                in_=sb_Wtmp[:, k * CD:(k + 1) * CD].rearrange(
                    "p (rp c) -> p rp c", c=C
                ),
                pattern=[[-8, 16], [0, C]],
                compare_op=mybir.AluOpType.is_ge,
                fill=0.0,
                base=0,
                channel_multiplier=1,
            )

        # t[b, (k, rp, c)] = sum_i Wd[(rp,i),(k,c)] u[b, r(k,rp), i]
        sb_t = [singles.tile([128, NCHUNK * CD], bf16, tag=f"t{bh}", name=f"t{bh}") for bh in range(2)]
        for k in range(NCHUNK):
            for bh in range(2):
                pt = ps_e.tile([128, CD], f32, tag="pe", name="pt")
                nc.tensor.matmul(
                    pt,
                    utb(k, bh),
                    sb_Wdbd[:, k * CD:(k + 1) * CD],
                    start=True, stop=True,
                )
                nc.vector.tensor_copy(out=sb_t[bh][:, k * CD:(k + 1) * CD], in_=pt)

        # b_ij tile: partitions (j, rp) with j = chunk % 3, free (g, c, d)
        sb_b = singles.tile([48, NG * CD], f32, tag="b")
        nc.vector.memset(sb_b, 0.0)
        sb_E = singles.tile([48, NG * CD], bf16, tag="E")
        sb_vb = singles.tile([128, 2 * CD], bf16, tag="vb")

        def warm_pe(tick, tag):
            """NWARM dummy LDWEIGHTS+MATMUL pairs whose lhsT reads `tick`
            (the first readback row-slice), so they start exactly when the
            collective completes and keep the PE array busy (HAM at K=8/8)
            through the readback + squash window; the real b-update matmuls
            then issue against a warm clock."""
            p = tick.partition_size()
            # The s0 PSUM bank is dead between iterations (its stage copy is
            # long done by the time the collective lands) - reuse it.
            junk = ps_s.tile([8, 128], f32, tag="s0", name=f"junk{tag}")
            for n in range(NWARM):
                nc.tensor.matmul(
                    junk, tick, sb_g[0][0:p, 0:128],
                    start=True, stop=True,
                )

        def squash(vf, y, z, zconst):
            """vf = squash(y/z) = y*|y| / (z^2 + y^2). The reference's
            eps=1e-5 guard only perturbs |y/z| ~ eps where v ~ 1e-10 -
            dropped. All DVE (GPSIMD sharing the SBUF ports measurably slows
            the concurrent DVE ops): y2 -> d1 -> rd chain, with |y| (one
            scalar_tensor_tensor op) and y*|y| slotted around it; z^2 (when
            present) is the only GPSIMD op."""
            p = y.partition_size()
            n2 = y.free_size()          # 2*CD
            if z is not None:
                z2 = work.tile([p, CD], f32, tag="z2", name="z2")
                nc.gpsimd.tensor_mul(z2, z, z)
            y2 = work.tile([p, n2], f32, tag="y2", name="y2")
            nc.vector.tensor_mul(y2, y, y)
            d1 = work.tile([p, n2], f32, tag="d1", name="d1")
            if z is None:
                nc.vector.tensor_scalar_add(d1, y2, float(zconst) ** 2)
            else:
                z2b = bass.AP(tensor=z2.tensor, offset=z2.offset,
                              ap=[z2.ap[0], [0, 2], [1, CD]])
                nc.vector.tensor_add(
                    d1.rearrange("p (h f) -> p h f", f=CD),
                    y2.rearrange("p (h f) -> p h f", f=CD),
                    z2b,
                )
            ay = work.tile([p, n2], f32, tag="ay", name="ay")
            nc.vector.scalar_tensor_tensor(ay, y, -1.0, y, ALU.mult, ALU.max)
            rd = work.tile([p, n2], f32, tag="rd", name="rd")
            nc.vector.reciprocal_approx_fast(out=rd, in_=d1)
            w = work.tile([p, n2], f32, tag="w", name="w")
            nc.vector.tensor_mul(w, y, ay)
            nc.vector.tensor_mul(vf, w, rd)

        for it in range(1, NITER):
            ci_p, co_p, w_p = cc[it - 1]
            # Split the collective readback row-wise across three DMA queues:
            # row slices keep each transfer contiguous in DRAM (full 2*w_p-byte
            # rows) instead of 128 strided descriptors per queue.
            red = stpool.tile([128, w_p], bf16, tag="red", name=f"red{it}")
            nc.sync.dma_start(out=red[0:64, :], in_=co_p[0:64, :])
            nc.scalar.dma_start(out=red[64:128, :], in_=co_p[64:128, :])
            warm_pe(red[0:64, 0:8], f"w{it}")

            # mid-iteration v is only ever consumed as a bf16 matmul operand,
            # so squash writes the bf16 tile directly (no copy).
            if it == 1:
                squash(sb_vb, red[:, 0:2 * CD], None, float(R))
            else:
                squash(sb_vb, red[:, 0:2 * CD], red[:, 2 * CD:3 * CD], None)

            # b[(j,rp), g*CD + c*D + d] += sum_b t[b,(g*3+j),rp,c] v[b, c*D+d]
            # The pe expand matmuls for group g are emitted after group g+1's
            # b-update so their exp(g) input is long since ready when they
            # reach the head of the in-order PE queue.
            t_r = [sb_t[bh].rearrange("p (k rp c) -> p k rp c", rp=16, c=C)
                   for bh in range(2)]
            pes = [ps_e.tile([128, NG * CD], f32, tag="pe", name=f"pe{it}{j}")
                   for j in range(NG)]

            def bupdate(g):
                pb = ps_b.tile([48, CD], f32, tag="pb")
                for c in range(C):
                    for bh in range(2):
                        nc.tensor.matmul(
                            pb[:, c * D:(c + 1) * D],
                            t_r[bh][:, g * NG:(g + 1) * NG, :, c],
                            sb_vb[:, bh * CD + c * D:bh * CD + (c + 1) * D],
                            start=(bh == 0), stop=(bh == 1),
                        )
                nc.vector.tensor_add(
                    sb_b[:, g * CD:(g + 1) * CD],
                    sb_b[:, g * CD:(g + 1) * CD],
                    pb,
                )
                nc.scalar.activation(
                    out=sb_E[:, g * CD:(g + 1) * CD],
                    in_=sb_b[:, g * CD:(g + 1) * CD],
                    func=AF.Exp,
                )

            def pe_expand(g):
                # pe_j[p, (g,cd)] = E[(j, p//8), (g,cd)] for this g only
                for j in range(NG):
                    nc.tensor.matmul(
                        pes[j][:, g * CD:(g + 1) * CD],
                        sb_exb[:, j * 128:(j + 1) * 128],
                        sb_E[:, g * CD:(g + 1) * CD],
                        start=True, stop=True,
                    )

            ci, co, w = cc[it]
            stage = stpool.tile([128, w], bf16, tag="stage", name=f"stage{it}")
            st = [ps_s.tile([128, CD], f32, tag=f"s{bh}", name=f"s{it}{bh}") for bh in range(2)]

            def rhs_muls(g):
                # E*W products for group g's three chunks (vector; emitted
                # right after pe_expand(g) so they overlap the next group's
                # b-update matmuls).
                out = []
                for j in range(NG):
                    k = g * NG + j
                    rhs = wcpool.tile([128, CD], bf16, tag="wc")
                    nc.vector.tensor_mul(
                        rhs, wtb(k), pes[j][:, g * CD:(g + 1) * CD]
                    )
                    out.append(rhs)
                return out

            def s_chunks(g, rhss, bh):
                for j in range(NG):
                    k = g * NG + j
                    nc.tensor.matmul(
                        st[bh], utb(k, bh), rhss[j],
                        start=(k == 0), stop=(k == NCHUNK - 1),
                    )

            # Interleaved schedule: group g's expand + E*W products + s'
            # matmuls slot between the later groups' b-updates, so the PE
            # queue never stalls on exp() and the s' stream is not paced by
            # the vector engine at the end of the iteration. The bh0 s'
            # accumulation completes before bh1's, so its stage copy + DMA
            # overlap bh1's matmul stream.
            bupdate(0)
            bupdate(1)
            pe_expand(0)
            rhss0 = rhs_muls(0)
            bupdate(2)
            pe_expand(1)
            rhss1 = rhs_muls(1)
            s_chunks(0, rhss0, 0)
            pe_expand(2)
            rhss2 = rhs_muls(2)
            s_chunks(1, rhss1, 0)

            pz = ps_z.tile([128, CD], f32, tag="pz", name="pz")
            for g in range(NG):
                nc.tensor.matmul(
                    pz, sb_ones, sb_E[:, g * CD:(g + 1) * CD],
                    start=(g == 0), stop=(g == NG - 1),
                )
            s_chunks(2, rhss2, 0)
            nc.scalar.copy(out=stage[:, 2 * CD:3 * CD], in_=pz)
            # Z rides the (slow but latency-tolerant) gpsimd SWDGE queue so
            # the sync/scalar HW queues stay free for the s' halves.
            nc.gpsimd.dma_start(out=ci[:, 2 * CD:3 * CD], in_=stage[:, 2 * CD:3 * CD])
            nc.scalar.copy(out=stage[:, 0:CD], in_=st[0])
            nc.sync.dma_start(out=ci[:, 0:CD], in_=stage[:, 0:CD])
            for g in range(NG):
                s_chunks(g, (rhss0, rhss1, rhss2)[g], 1)
            nc.vector.tensor_copy(out=stage[:, CD:2 * CD], in_=st[1])
            nc.scalar.dma_start(out=ci[:, CD:2 * CD], in_=stage[:, CD:2 * CD])

            if it < NITER - 1:
                nc.gpsimd.collective_compute(
                    "AllReduce", mybir.AluOpType.add,
                    replica_groups=rgroups, ins=[ci.opt()], outs=[co.opt()],
                )
            else:
                # Final: ReduceScatter - each core keeps only its 16-partition
                # shard (batches 16c..16c+16 of each b-half) and outputs it.
                nc.gpsimd.collective_compute(
                    "ReduceScatter", mybir.AluOpType.add,
                    replica_groups=rgroups, ins=[ci.opt()], outs=[co.opt()],
                )
                red_f = stpool.tile([RSP, w], bf16, tag="redf", name="redf")
                nc.sync.dma_start(out=red_f[0:RSP // 2, :], in_=co[0:RSP // 2, :])
                nc.scalar.dma_start(out=red_f[RSP // 2:RSP, :], in_=co[RSP // 2:RSP, :])
                vf = work.tile([RSP, 2 * CD], f32, tag="vff", name="vff")
                squash(vf, red_f[:, 0:2 * CD], red_f[:, 2 * CD:3 * CD], None)
                nc.sync.dma_start(out=out_d[:], in_=vf)

    nc.compile()
    return nc


def _host_inputs(u, W):
    """Host prep: per-core (r,i)-major chunk-group layouts, shipped as bf16
    (the kernel's matmul operands are bf16 regardless; rounding on the host
    is numerically identical and halves the feed)."""
    import ml_dtypes

    bf = ml_dtypes.bfloat16
    u = np.ascontiguousarray(u, dtype=np.float32)
    W = np.ascontiguousarray(W, dtype=np.float32)
    DW = NG * GW
    in_maps = []
    for ci in range(NCORES):
        rs = ci * RL
        usl = u[:, rs:rs + RL, :].reshape(B, RL * I).T          # (1152, 256)
        uTd = usl.reshape(NCHUNK, 128, B).astype(bf)
        wsl = W[rs:rs + RL].transpose(0, 3, 1, 2).reshape(RL * I, CD)
        Wtd = wsl.reshape(NCHUNK, 128, CD).astype(bf)
        dat = np.zeros((128, DW), dtype=bf)
        for g in range(NG):
            o = g * GW
            for j in range(NG):
                k = g * NG + j
                dat[:, o + j * B:o + (j + 1) * B] = uTd[k]
                dat[:, o + NG * B + j * CD:o + NG * B + (j + 1) * CD] = Wtd[k]
        in_maps.append({"data": dat})
    return in_maps


def _install_profile_hook():
    """Recreate the missing antenv.axon_hooks NTFF-profile hook (dev only)."""
    import contextlib
    import ctypes
    import sys
    import types

    try:
        from antenv.axon_hooks import get_axon_ntff_profile_hook  # noqa: F401
        return
    except ImportError:
        pass

    mod = types.ModuleType("antenv.axon_hooks")
    holder = {}
    mod.set_axon_ntff_profile_hook = lambda h: holder.__setitem__("h", h)
    mod.get_axon_ntff_profile_hook = lambda: holder.get("h")
    import antenv

    sys.modules["antenv.axon_hooks"] = mod
    antenv.axon_hooks = mod

    so_path = "/opt/axon/libaxon_pjrt.so"
    lib = ctypes.CDLL(so_path)
    if not hasattr(lib, "axon_start_nrt_profile"):
        return
    lib.axon_start_nrt_profile.argtypes = [
        ctypes.POINTER(ctypes.c_int64),
        ctypes.c_size_t,
    ]
    lib.axon_start_nrt_profile.restype = ctypes.c_int64
    lib.axon_stop_nrt_profile.argtypes = [ctypes.c_char_p]
    lib.axon_stop_nrt_profile.restype = ctypes.c_int64

    @contextlib.contextmanager
    def _hook(output_dir, device_ids):
        import jax

        jax.devices()
        if device_ids:
            ids = (ctypes.c_int64 * len(device_ids))(*device_ids)
            rc = lib.axon_start_nrt_profile(ids, len(device_ids))
        else:
            rc = lib.axon_start_nrt_profile(None, 0)
        if rc != 0:
            raise RuntimeError(f"axon_start_nrt_profile rc={rc}")
        try:
            yield
        finally:
            n = lib.axon_stop_nrt_profile(str(output_dir).encode())
            print(f"profile: {n} file(s) written to {output_dir}")

    mod.set_axon_ntff_profile_hook(_hook)

    # Avoid the bucket upload inside the trace post-processing.
    import concourse.bass_utils as bu

    bu.upload_artifacts = lambda tmpdir: f"local:{tmpdir}"


def kernel(u, W):
    from concourse.bass_utils import run_bass_kernel_spmd

    if os.environ.get("KERNEL_TRACE", "0") == "1":
        _install_profile_hook()
    if "nc" not in _CACHE:
        _CACHE["nc"] = _build_program()
    nc = _CACHE["nc"]
    in_maps = _host_inputs(u, W)
    trace = os.environ.get("KERNEL_TRACE", "0") == "1"
    res = run_bass_kernel_spmd(
        nc, in_maps, core_ids=list(range(NCORES)), trace=trace
    )
    _CACHE["last_result"] = res
    out = np.zeros((B, CD), dtype=np.float32)
    for k in range(NCORES):
        o = np.asarray(res.results[k]["out"])          # [RSP, 2*CD]
        out[RSP * k:RSP * (k + 1)] = o[:, 0:CD]
        out[128 + RSP * k:128 + RSP * (k + 1)] = o[:, CD:2 * CD]
    return out.reshape(B, C, D)


# revision 23
# speedup vs baseline: 1.0326x; 1.0326x over previous
"""DigitCaps dynamic-routing kernel for 8 TRN2 NeuronCores.

Math refactor (u_hat is NEVER materialized - it would be 189 MB):
  u_hat[b,r,c,d] = sum_i W[r,c,d,i] * u[b,r,i]
  softmax over r without max-subtraction (b_ij values are O(1)):
      c_ij[r,c,d] = exp(b[r,c,d]) / Z[c,d],  Z = sum_r exp(b)
  s'[b,c,d] = sum_{r,i} (exp(b) * W)[r,c,d,i] u[b,r,i],  s = s'/Z
  v = squash(s) = squash(y/z) computed WITHOUT the division:
      v = y^3 / ((z^2 + y^2) * (|y| + eps*z))        (y = s', z = Z)
  b += (1/B) sum_b t[b,r,c] v[b,c,d],  t[b,r,c] = sum_i (sum_d W) u[b,r,i]
       (t is iteration-invariant -> computed once, during AllReduce 0)

Sharding: routes (R=1152) split across 8 cores (144 each). Iterations 0 and 1
end in a fused AllReduce of (partial s' || partial Z); iteration 2 ends in a
ReduceScatter so each core only computes/outputs v for its own 32 batches
(host concatenates the disjoint shards - pure gather).

Schedule notes (v2):
  - Collective readback is split across three DMA queues (sync/scalar/gpsimd)
    so the 120KB payload lands in ~1us instead of ~2.5us.
  - A short burst of dummy matmuls (LDWEIGHTS reads the first readback slice,
    so they cannot start before the collective completes) keeps the PE busy
    through the readback+squash window: the HAM clock gate reaches K=8/8
    before the 60 b-update matmuls issue, halving their pacing.
  - squash runs split across DVE (y^2 -> denom -> recip -> final mul) and
    GPSIMD (|y| via abs_max, y*|y|), with |y| as a single abs_max op.
  - pe expand matmuls are emitted per (j, group) and pipelined between the
    per-group b-update matmuls, so the E*W rhs products (on GPSIMD) overlap
    the b-update phase and the s' matmul stream is not vector-paced.
  - stage copies go scalar (bh0) + vector (bh1) in parallel; the staging DMAs
    to the collective input split across two queues.
"""

import os
import numpy as np

B, R, C, D, I = 256, 1152, 10, 16, 8
CD = C * D                 # 160
NCORES = 8
RL = R // NCORES           # 144 routes per core
NCHUNK = RL * I // 128     # 9 K-chunks of 128
NG = 3                     # chunk groups of 3 (b_ij tile partition packing)
NITER = 3
EPS = 1e-5
GW = NG * (B + CD)         # columns per chunk-group input tile (1248)
RSP = 128 // NCORES        # 16 partitions per core after ReduceScatter
NWARM = 28                 # keep-warm dummy matmul pairs per collective wait

_CACHE = {}


def _build_program():
    from contextlib import ExitStack

    import concourse.bass as bass
    import concourse.bacc as bacc
    import concourse.mybir as mybir
    import concourse.tile as tile

    f32 = mybir.dt.float32
    bf16 = mybir.dt.bfloat16
    AF = mybir.ActivationFunctionType
    ALU = mybir.AluOpType

    nc = bacc.Bacc(None, num_devices=NCORES)

    # One fused host parameter (a single H2D feed per core keeps launch skew
    # across cores minimal) but THREE device-side DMAs over its column ranges
    # so the iter-0 matmuls pipeline with the load. The payload is bf16 -
    # numerically identical to casting on device (all matmul operands are
    # bf16 anyway) at half the H2D/DMA bytes. The 0/1 selector constants
    # (expand / route mask) are generated on-device with affine_select.
    # Layout: [g0 | g1 | g2], each g = [uT_k x3 | Wt_k x3].
    DW = NG * GW
    data_d = nc.declare_dram_parameter("data", [128, DW], bf16, isOutput=False)
    out_d = nc.declare_dram_parameter("out", [RSP, 2 * CD], f32, isOutput=True)

    rgroups = [list(range(NCORES))]

    with tile.TileContext(nc) as tc, ExitStack() as ctx:
        singles = ctx.enter_context(tc.tile_pool(name="singles", bufs=1))
        wcpool = ctx.enter_context(tc.tile_pool(name="wc", bufs=9))
        stpool = ctx.enter_context(tc.tile_pool(name="stage", bufs=2))
        work = ctx.enter_context(tc.tile_pool(name="work", bufs=8))
        ps_s = ctx.enter_context(tc.tile_pool(name="ps_s", bufs=1, space="PSUM"))
        ps_e = ctx.enter_context(tc.tile_pool(name="ps_e", bufs=3, space="PSUM"))
        ps_z = ctx.enter_context(tc.tile_pool(name="ps_z", bufs=1, space="PSUM"))
        ps_b = ctx.enter_context(tc.tile_pool(name="ps_b", bufs=2, space="PSUM"))
        dram = ctx.enter_context(tc.tile_pool(name="dram", bufs=1, space="DRAM"))

        # Collective payloads in bf16: halves the wire bytes and the
        # staging/readback DMAs (CCE reduces in bf16; ~0.4% noise, well
        # inside the tolerance).
        cc = []
        for it in range(NITER):
            w = 2 * CD if it == 0 else 3 * CD
            po = RSP if it == NITER - 1 else 128   # RS output is a partition shard
            ci = dram.tile([128, w], bf16, tag=f"cc_in{it}", name=f"cc_in{it}")
            co = dram.tile([po, w], bf16, tag=f"cc_out{it}", name=f"cc_out{it}")
            cc.append((ci, co, w))

        # ---- input DMAs (issued immediately; matmuls chase them) ----
        sb_g = []
        for g in range(NG):
            t = singles.tile([128, GW], bf16, tag=f"g{g}", name=f"sbg{g}")
            nc.sync.dma_start(out=t, in_=data_d[:, g * GW:(g + 1) * GW])
            sb_g.append(t)

        def wtb(k):
            g, j = k // NG, k % NG
            return sb_g[g][:, NG * B + j * CD:NG * B + (j + 1) * CD]

        def utb(k, bh):
            g, j = k // NG, k % NG
            return sb_g[g][:, j * B + bh * 128:j * B + (bh + 1) * 128]

        # ---- iteration 0 partial s' (c_ij uniform): straight to AllReduce ----
        # b-halves sequenced so bh0's staging DMA overlaps bh1's matmuls.
        ci0, co0, w0 = cc[0]
        stage0 = stpool.tile([128, w0], bf16, tag="stage", name="stage0")
        st = [ps_s.tile([128, CD], f32, tag=f"s{bh}", name=f"s0{bh}") for bh in range(2)]
        for bh in range(2):
            for k in range(NCHUNK):
                nc.tensor.matmul(
                    st[bh],
                    utb(k, bh),
                    wtb(k),
                    start=(k == 0), stop=(k == NCHUNK - 1),
                )
            nc.scalar.copy(out=stage0[:, bh * CD:(bh + 1) * CD], in_=st[bh])
        nc.sync.dma_start(out=ci0[:], in_=stage0)
        nc.gpsimd.collective_compute(
            "AllReduce", mybir.AluOpType.add,
            replica_groups=rgroups, ins=[ci0.opt()], outs=[co0.opt()],
        )

        # ---- prep work that hides under AllReduce 0 ----
        sb_ones = singles.tile([48, 128], bf16, tag="ones")
        nc.vector.memset(sb_ones, 1.0)

        # expand[q, j*128 + a*8 + b] = (q == 16j + a): the partition-broadcast
        # selector for the pe matmuls, generated on-device (GPSIMD, idle here).
        sb_exb = singles.tile([48, NG * 128], bf16, tag="exb")
        for j in range(NG):
            nc.gpsimd.affine_select(
                out=sb_exb[:, j * 128:(j + 1) * 128].rearrange(
                    "q (a b) -> q a b", b=8
                ),
                in_=sb_ones.rearrange("q (a b) -> q a b", b=8),
                pattern=[[-1, 16], [0, 8]],
                compare_op=mybir.AluOpType.is_equal,
                fill=0.0,
                base=-16 * j,
                channel_multiplier=1,
            )

        # Wd[(rp,i), (k,c)] = (1/B) * sum_d Wt   (t pre-scaled by 1/B here)
        sb_Wd = singles.tile([128, NCHUNK * C], f32, tag="Wd")
        for k in range(NCHUNK):
            nc.vector.reduce_sum(
                out=sb_Wd[:, k * C:(k + 1) * C],
                in_=wtb(k).rearrange("p (c d) -> p c d", d=D),
                axis=mybir.AxisListType.X,
            )
        nc.vector.tensor_scalar_mul(sb_Wd, sb_Wd, 1.0 / B)

        # Block-diagonal Wd for the t matmul:
        # Wdbd[p, k*CD + rp*C + c] = Wd[p,(k,c)] * (rp == p//8),
        # built as two affine_selects (p <= 8*rp+7 then p >= 8*rp) over a
        # zero-stride broadcast of Wd - no mask constant needed.
        sb_Wdbd = singles.tile([128, NCHUNK * CD], bf16, tag="Wdbd")
        sb_Wtmp = singles.tile([128, NCHUNK * CD], bf16, tag="Wtmp")
        for k in range(NCHUNK):
            wd_sl = sb_Wd[:, k * C:(k + 1) * C]
            wd_bk = bass.AP(
                tensor=wd_sl.tensor, offset=wd_sl.offset,
                ap=[wd_sl.ap[0], [0, 16], [1, C]],
            )
            nc.gpsimd.affine_select(
                out=sb_Wtmp[:, k * CD:(k + 1) * CD].rearrange(
                    "p (rp c) -> p rp c", c=C
                ),
                in_=wd_bk,
                pattern=[[8, 16], [0, C]],
                compare_op=mybir.AluOpType.is_ge,
                fill=0.0,
                base=7,
                channel_multiplier=-1,
            )
            nc.gpsimd.affine_select(
                out=sb_Wdbd[:, k * CD:(k + 1) * CD].rearrange(
                    "p (rp c) -> p rp c", c=C
                ),
                in_=sb_Wtmp[:, k * CD:(k + 1) * CD].rearrange(
                    "p (rp c) -> p rp c", c=C
                ),
                pattern=[[-8, 16], [0, C]],
                compare_op=mybir.AluOpType.is_ge,
                fill=0.0,
                base=0,
                channel_multiplier=1,
            )

        # t[b, (k, rp, c)] = sum_i Wd[(rp,i),(k,c)] u[b, r(k,rp), i]
        sb_t = [singles.tile([128, NCHUNK * CD], bf16, tag=f"t{bh}", name=f"t{bh}") for bh in range(2)]
        for k in range(NCHUNK):
            for bh in range(2):
                pt = ps_e.tile([128, CD], f32, tag="pe", name="pt")
                nc.tensor.matmul(
                    pt,
                    utb(k, bh),
                    sb_Wdbd[:, k * CD:(k + 1) * CD],
                    start=True, stop=True,
                )
                nc.vector.tensor_copy(out=sb_t[bh][:, k * CD:(k + 1) * CD], in_=pt)

        # b_ij tile: partitions (j, rp) with j = chunk % 3, free (g, c, d)
        sb_b = singles.tile([48, NG * CD], f32, tag="b")
        nc.vector.memset(sb_b, 0.0)
        sb_E = singles.tile([48, NG * CD], bf16, tag="E")
        sb_vb = singles.tile([128, 2 * CD], bf16, tag="vb")

        def warm_pe(tick, tag):
            """NWARM dummy LDWEIGHTS+MATMUL pairs whose lhsT reads `tick`
            (the first readback row-slice), so they start exactly when the
            collective completes and keep the PE array busy (HAM at K=8/8)
            through the readback + squash window; the real b-update matmuls
            then issue against a warm clock."""
            p = tick.partition_size()
            # The s0 PSUM bank is dead between iterations (its stage copy is
            # long done by the time the collective lands) - reuse it.
            junk = ps_s.tile([8, 128], f32, tag="s0", name=f"junk{tag}")
            for n in range(NWARM):
                nc.tensor.matmul(
                    junk, tick, sb_g[0][0:p, 0:128],
                    start=True, stop=True,
                )

        def squash(vf, y, z, zconst):
            """vf = squash(y/z) = y*|y| / (z^2 + y^2). The reference's
            eps=1e-5 guard only perturbs |y/z| ~ eps where v ~ 1e-10 -
            dropped. All DVE (GPSIMD sharing the SBUF ports measurably slows
            the concurrent DVE ops): y2 -> d1 -> rd chain, with |y| (one
            scalar_tensor_tensor op) and y*|y| slotted around it; z^2 (when
            present) is the only GPSIMD op."""
            p = y.partition_size()
            n2 = y.free_size()          # 2*CD
            if z is not None:
                z2 = work.tile([p, CD], f32, tag="z2", name="z2")
                nc.gpsimd.tensor_mul(z2, z, z)
            y2 = work.tile([p, n2], f32, tag="y2", name="y2")
            nc.vector.tensor_mul(y2, y, y)
            d1 = work.tile([p, n2], f32, tag="d1", name="d1")
            if z is None:
                nc.vector.tensor_scalar_add(d1, y2, float(zconst) ** 2)
            else:
                z2b = bass.AP(tensor=z2.tensor, offset=z2.offset,
                              ap=[z2.ap[0], [0, 2], [1, CD]])
                nc.vector.tensor_add(
                    d1.rearrange("p (h f) -> p h f", f=CD),
                    y2.rearrange("p (h f) -> p h f", f=CD),
                    z2b,
                )
            ay = work.tile([p, n2], f32, tag="ay", name="ay")
            nc.vector.scalar_tensor_tensor(ay, y, -1.0, y, ALU.mult, ALU.max)
            rd = work.tile([p, n2], f32, tag="rd", name="rd")
            nc.vector.reciprocal_approx_fast(out=rd, in_=d1)
            w = work.tile([p, n2], f32, tag="w", name="w")
            nc.vector.tensor_mul(w, y, ay)
            nc.vector.tensor_mul(vf, w, rd)

        for it in range(1, NITER):
            ci_p, co_p, w_p = cc[it - 1]
            # Split the collective readback row-wise across three DMA queues:
            # row slices keep each transfer contiguous in DRAM (full 2*w_p-byte
            # rows) instead of 128 strided descriptors per queue.
            red = stpool.tile([128, w_p], bf16, tag="red", name=f"red{it}")
            nc.sync.dma_start(out=red[0:64, :], in_=co_p[0:64, :])
            nc.scalar.dma_start(out=red[64:128, :], in_=co_p[64:128, :])
            warm_pe(red[0:64, 0:8], f"w{it}")

            # mid-iteration v is only ever consumed as a bf16 matmul operand,
            # so squash writes the bf16 tile directly (no copy).
            if it == 1:
                squash(sb_vb, red[:, 0:2 * CD], None, float(R))
            else:
                squash(sb_vb, red[:, 0:2 * CD], red[:, 2 * CD:3 * CD], None)

            # b[(j,rp), g*CD + c*D + d] += sum_b t[b,(g*3+j),rp,c] v[b, c*D+d]
            # The pe expand matmuls for group g are emitted after group g+1's
            # b-update so their exp(g) input is long since ready when they
            # reach the head of the in-order PE queue.
            t_r = [sb_t[bh].rearrange("p (k rp c) -> p k rp c", rp=16, c=C)
                   for bh in range(2)]
            pes = [ps_e.tile([128, NG * CD], f32, tag="pe", name=f"pe{it}{j}")
                   for j in range(NG)]

            def bupdate(g):
                pb = ps_b.tile([48, CD], f32, tag="pb")
                for c in range(C):
                    for bh in range(2):
                        nc.tensor.matmul(
                            pb[:, c * D:(c + 1) * D],
                            t_r[bh][:, g * NG:(g + 1) * NG, :, c],
                            sb_vb[:, bh * CD + c * D:bh * CD + (c + 1) * D],
                            start=(bh == 0), stop=(bh == 1),
                        )
                nc.vector.tensor_add(
                    sb_b[:, g * CD:(g + 1) * CD],
                    sb_b[:, g * CD:(g + 1) * CD],
                    pb,
                )
                nc.scalar.activation(
                    out=sb_E[:, g * CD:(g + 1) * CD],
                    in_=sb_b[:, g * CD:(g + 1) * CD],
                    func=AF.Exp,
                )

            def pe_expand(g):
                # pe_j[p, (g,cd)] = E[(j, p//8), (g,cd)] for this g only
                for j in range(NG):
                    nc.tensor.matmul(
                        pes[j][:, g * CD:(g + 1) * CD],
                        sb_exb[:, j * 128:(j + 1) * 128],
                        sb_E[:, g * CD:(g + 1) * CD],
                        start=True, stop=True,
                    )

            ci, co, w = cc[it]
            stage = stpool.tile([128, w], bf16, tag="stage", name=f"stage{it}")
            st = [ps_s.tile([128, CD], f32, tag=f"s{bh}", name=f"s{it}{bh}") for bh in range(2)]

            def rhs_muls(g):
                # E*W products for group g's three chunks (vector; emitted
                # right after pe_expand(g) so they overlap the next group's
                # b-update matmuls).
                out = []
                for j in range(NG):
                    k = g * NG + j
                    rhs = wcpool.tile([128, CD], bf16, tag="wc")
                    nc.vector.tensor_mul(
                        rhs, wtb(k), pes[j][:, g * CD:(g + 1) * CD]
                    )
                    out.append(rhs)
                return out

            def s_chunks(g, rhss, bh):
                for j in range(NG):
                    k = g * NG + j
                    nc.tensor.matmul(
                        st[bh], utb(k, bh), rhss[j],
                        start=(k == 0), stop=(k == NCHUNK - 1),
                    )

            # Interleaved schedule: group g's expand + E*W products + s'
            # matmuls slot between the later groups' b-updates, so the PE
            # queue never stalls on exp() and the s' stream is not paced by
            # the vector engine at the end of the iteration. The bh0 s'
            # accumulation completes before bh1's, so its stage copy + DMA
            # overlap bh1's matmul stream.
            bupdate(0)
            bupdate(1)
            pe_expand(0)
            rhss0 = rhs_muls(0)
            bupdate(2)
            pe_expand(1)
            rhss1 = rhs_muls(1)
            s_chunks(0, rhss0, 0)
            pe_expand(2)
            rhss2 = rhs_muls(2)
            s_chunks(1, rhss1, 0)

            pz = ps_z.tile([128, CD], f32, tag="pz", name="pz")
            for g in range(NG):
                nc.tensor.matmul(
                    pz, sb_ones, sb_E[:, g * CD:(g + 1) * CD],
                    start=(g == 0), stop=(g == NG - 1),
                )
            s_chunks(2, rhss2, 0)
            nc.scalar.copy(out=stage[:, 2 * CD:3 * CD], in_=pz)
            # Z rides the (slow but latency-tolerant) gpsimd SWDGE queue so
            # the sync/scalar HW queues stay free for the s' halves.
            nc.gpsimd.dma_start(out=ci[:, 2 * CD:3 * CD], in_=stage[:, 2 * CD:3 * CD])
            nc.scalar.copy(out=stage[:, 0:CD], in_=st[0])
            nc.sync.dma_start(out=ci[:, 0:CD], in_=stage[:, 0:CD])
            for g in range(NG):
                s_chunks(g, (rhss0, rhss1, rhss2)[g], 1)
            nc.vector.tensor_copy(out=stage[:, CD:2 * CD], in_=st[1])
            nc.scalar.dma_start(out=ci[:, CD:2 * CD], in_=stage[:, CD:2 * CD])

            if it < NITER - 1:
                nc.gpsimd.collective_compute(
                    "AllReduce", mybir.AluOpType.add,
                    replica_groups=rgroups, ins=[ci.opt()], outs=[co.opt()],
                )
            else:
                # Final: ReduceScatter - each core keeps only its 16-partition
                # shard (batches 16c..16c+16 of each b-half) and outputs it.
                nc.gpsimd.collective_compute(
                    "ReduceScatter", mybir.AluOpType.add,
                    replica_groups=rgroups, ins=[ci.opt()], outs=[co.opt()],
                )
                red_f = stpool.tile([RSP, w], bf16, tag="redf", name="redf")
                nc.sync.dma_start(out=red_f[0:RSP // 2, :], in_=co[0:RSP // 2, :])
                nc.scalar.dma_start(out=red_f[RSP // 2:RSP, :], in_=co[RSP // 2:RSP, :])
                vf = work.tile([RSP, 2 * CD], f32, tag="vff", name="vff")
                squash(vf, red_f[:, 0:2 * CD], red_f[:, 2 * CD:3 * CD], None)
                nc.sync.dma_start(out=out_d[:], in_=vf)

    nc.compile()
    return nc


def _host_inputs(u, W):
    """Host prep: per-core (r,i)-major chunk-group layouts, shipped as bf16
    (the kernel's matmul operands are bf16 regardless; rounding on the host
    is numerically identical and halves the feed)."""
    import ml_dtypes

    bf = ml_dtypes.bfloat16
    u = np.ascontiguousarray(u, dtype=np.float32)
    W = np.ascontiguousarray(W, dtype=np.float32)
    DW = NG * GW
    in_maps = []
    for ci in range(NCORES):
        rs = ci * RL
        usl = u[:, rs:rs + RL, :].reshape(B, RL * I).T          # (1152, 256)
        uTd = usl.reshape(NCHUNK, 128, B).astype(bf)
        wsl = W[rs:rs + RL].transpose(0, 3, 1, 2).reshape(RL * I, CD)
        Wtd = wsl.reshape(NCHUNK, 128, CD).astype(bf)
        dat = np.zeros((128, DW), dtype=bf)
        for g in range(NG):
            o = g * GW
            for j in range(NG):
                k = g * NG + j
                dat[:, o + j * B:o + (j + 1) * B] = uTd[k]
                dat[:, o + NG * B + j * CD:o + NG * B + (j + 1) * CD] = Wtd[k]
        in_maps.append({"data": dat})
    return in_maps


def _install_profile_hook():
    """Recreate the missing antenv.axon_hooks NTFF-profile hook (dev only)."""
    import contextlib
    import ctypes
    import sys
    import types

    try:
        from antenv.axon_hooks import get_axon_ntff_profile_hook  # noqa: F401
        return
    except ImportError:
        pass

    mod = types.ModuleType("antenv.axon_hooks")
    holder = {}
    mod.set_axon_ntff_profile_hook = lambda h: holder.__setitem__("h", h)
    mod.get_axon_ntff_profile_hook = lambda: holder.get("h")
    import antenv

    sys.modules["antenv.axon_hooks"] = mod
    antenv.axon_hooks = mod

    so_path = "/opt/axon/libaxon_pjrt.so"
    lib = ctypes.CDLL(so_path)
    if not hasattr(lib, "axon_start_nrt_profile"):
        return
    lib.axon_start_nrt_profile.argtypes = [
        ctypes.POINTER(ctypes.c_int64),
        ctypes.c_size_t,
    ]
    lib.axon_start_nrt_profile.restype = ctypes.c_int64
    lib.axon_stop_nrt_profile.argtypes = [ctypes.c_char_p]
    lib.axon_stop_nrt_profile.restype = ctypes.c_int64

    @contextlib.contextmanager
    def _hook(output_dir, device_ids):
        import jax

        jax.devices()
        if device_ids:
            ids = (ctypes.c_int64 * len(device_ids))(*device_ids)
            rc = lib.axon_start_nrt_profile(ids, len(device_ids))
        else:
            rc = lib.axon_start_nrt_profile(None, 0)
        if rc != 0:
            raise RuntimeError(f"axon_start_nrt_profile rc={rc}")
        try:
            yield
        finally:
            n = lib.axon_stop_nrt_profile(str(output_dir).encode())
            print(f"profile: {n} file(s) written to {output_dir}")

    mod.set_axon_ntff_profile_hook(_hook)

    # Avoid the bucket upload inside the trace post-processing.
    import concourse.bass_utils as bu

    bu.upload_artifacts = lambda tmpdir: f"local:{tmpdir}"


def kernel(u, W):
    from concourse.bass_utils import run_bass_kernel_spmd

    if os.environ.get("KERNEL_TRACE", "0") == "1":
        _install_profile_hook()
    if "nc" not in _CACHE:
        _CACHE["nc"] = _build_program()
    nc = _CACHE["nc"]
    in_maps = _host_inputs(u, W)
    trace = os.environ.get("KERNEL_TRACE", "0") == "1"
    res = run_bass_kernel_spmd(
        nc, in_maps, core_ids=list(range(NCORES)), trace=trace
    )
    _CACHE["last_result"] = res
    out = np.zeros((B, CD), dtype=np.float32)
    for k in range(NCORES):
        o = np.asarray(res.results[k]["out"])          # [RSP, 2*CD]
        out[RSP * k:RSP * (k + 1)] = o[:, 0:CD]
        out[128 + RSP * k:128 + RSP * (k + 1)] = o[:, CD:2 * CD]
    return out.reshape(B, C, D)
